# revision 1
# baseline (speedup 1.0000x reference)
"""Trainium2 Bass kernel for the Gaussian-span multi-head self-attention module.

  span  = head_reshape(h @ W_span.T, 2)          (B*K, M, 2)
  value = head_reshape(h @ W_val.T, D)           (B*K, M, D)
  mean  = sigmoid(span0) * M ; soft = softplus(span1)
  attn  = softmax(-soft * (pos - mean)^2)        (B*K, M, M)
  out   = (attn @ value)  -> concat heads -> @ W_out.T

Shapes are hardcoded: B=2, M=2048, HS=1024, K=16 heads, D=64.

Strategy (8 NeuronCores, SPMD — one program, per-core data):
  * batch*head sharding: core = b*4 + g handles batch b, heads [4g, 4g+4).
  * The Gaussian attention rows are extremely localized (soft >= ~0.01 means the
    window exp(-soft*(j-mean)^2) dies within |j-mean| <~ sqrt(50/soft)).  The host
    computes mean/soft (the tiny span projection, 0.25% of total FLOPs), sorts each
    head's query rows by mean, and builds a windowed schedule: for each 128-wide
    key block jb, only the contiguous range of sorted rows whose window touches it
    is processed (~9x fewer score elements than dense).  The schedule is the
    envelope over all 32 head-instances, so all 8 cores share one NEFF.
  * Scores are produced on the TensorEngine as a rank-3 matmul (basis [u^2, u, 1]
    centered per key block x host-precomputed coefficients [s, -2 s t, s t^2]),
    then a single ScalarEngine exp(-x) pass yields attention weights in bf16.
  * attn @ value accumulates out^T (65 x M) in PSUM with the value tile as the
    stationary operand; a ones-column in the value tile produces the softmax
    denominator for free.  Normalization uses a K=1 broadcast matmul + DVE mult.
  * The sorted->natural row un-permutation runs on the otherwise-idle GPSIMD
    engine via local_scatter on (head-pair x M) bf16 tiles.
  * Output projection is a bf16 matmul per 128-row block; per-core partials
    (one per batch half) are summed on the host.
"""

import sys
import types

import numpy as np
import ml_dtypes

B, M, HS, NH, D = 2, 2048, 1024, 16, 64
NCORES = 8
HPC = 4            # heads per core
CP = HPC * D       # 256-wide channel slice per core
SUB = 512          # scores i sub-chunk
NJB = M // 128     # key blocks
TAIL_T = 30.0      # window cut: exp(-TAIL_T) ~ 9e-14

_CACHE = {}        # ranges tuple -> compiled Bass program


def _ensure_ntff_hook():
    """Install the antenv.axon_hooks shim if the image lacks it (profiling only)."""
    try:
        import antenv.axon_hooks  # noqa: F401
        return
    except ImportError:
        pass
    try:
        import antenv
        from trn_agent_boot.trn_boot import _ntff_profile_via_ctypes
    except ImportError:
        return
    mod = types.ModuleType("antenv.axon_hooks")
    _h = [None]
    mod.set_axon_ntff_profile_hook = lambda hk: _h.__setitem__(0, hk)
    mod.get_axon_ntff_profile_hook = lambda: _h[0]
    sys.modules["antenv.axon_hooks"] = mod
    antenv.axon_hooks = mod
    try:
        mod.set_axon_ntff_profile_hook(
            _ntff_profile_via_ctypes("/opt/axon/libaxon_pjrt.so"))
    except Exception:
        pass


def _sigmoid64(x):
    return 1.0 / (1.0 + np.exp(-x.astype(np.float64)))


def _softplus64(x):
    return np.logaddexp(0.0, x.astype(np.float64))


def _build_host_data(h, W_span, W_val, W_out):
    h = np.asarray(h, np.float32)
    W_span = np.asarray(W_span, np.float32)
    W_val = np.asarray(W_val, np.float32)
    W_out = np.asarray(W_out, np.float32)

    span = (h.reshape(B * M, HS) @ W_span.T).reshape(B, M, 2 * NH)

    m_all = np.zeros((B, NH, M), np.float64)
    s_all = np.zeros((B, NH, M), np.float64)
    for b in range(B):
        for k in range(NH):
            m_all[b, k] = _sigmoid64(span[b, :, 2 * k]) * M
            s_all[b, k] = _softplus64(span[b, :, 2 * k + 1])
    order_all = np.argsort(m_all, axis=-1, kind="stable")
    W_all = np.sqrt(TAIL_T / np.maximum(s_all, 1e-12))

    ilos = np.full(NJB, M, np.int64)
    ihis = np.zeros(NJB, np.int64)
    for b in range(B):
        for k in range(NH):
            ms = m_all[b, k][order_all[b, k]]
            ws = W_all[b, k][order_all[b, k]]
            lo, hi = ms - ws, ms + ws
            for jb in range(NJB):
                mask = (hi >= jb * 128) & (lo <= jb * 128 + 128)
                idx = np.flatnonzero(mask)
                if idx.size:
                    ilos[jb] = min(ilos[jb], idx[0])
                    ihis[jb] = max(ihis[jb], idx[-1] + 1)
    ranges = []
    for jb in range(NJB):
        if ihis[jb] <= ilos[jb]:
            ranges.append((0, 0))
        else:
            ranges.append((int(ilos[jb]) & ~7, min(M, (int(ihis[jb]) + 7) & ~7)))

    # coverage: every sorted row must fall in the range of its own mean's block
    for b in range(B):
        for k in range(NH):
            ms = m_all[b, k][order_all[b, k]]
            own = np.clip((ms // 128).astype(np.int64), 0, NJB - 1)
            pos = np.arange(M)
            lows = np.array([ranges[j][0] for j in own])
            highs = np.array([ranges[j][1] for j in own])
            if not ((lows <= pos) & (pos < highs)).all():
                raise AssertionError("window schedule does not cover all rows")

    offs, cw = [], 0
    for lo, hi in ranges:
        offs.append(cw)
        cw += hi - lo

    in_maps = []
    for core in range(NCORES):
        b, g = core // HPC, core % HPC
        heads = [g * HPC + kk for kk in range(HPC)]

        hTb = np.ascontiguousarray(h[b].T).astype(ml_dtypes.bfloat16)
        Wv = np.ascontiguousarray(W_val[g * CP:(g + 1) * CP, :].T).astype(ml_dtypes.bfloat16)
        Wo = np.ascontiguousarray(W_out[:, g * CP:(g + 1) * CP].T).astype(ml_dtypes.bfloat16)

        A3 = np.zeros((HPC, 3, cw), np.float32)
        sidx = np.zeros((2, 2, 128, M), np.int16)
        for kk, k in enumerate(heads):
            order = order_all[b, k]
            ms = m_all[b, k][order]
            ss = s_all[b, k][order]
            for jb in range(NJB):
                lo, hi = ranges[jb]
                if hi <= lo:
                    continue
                t = ms[lo:hi] - (128.0 * jb + 64.0)
                s_ = ss[lo:hi]
                o = offs[jb]
                A3[kk, 0, o:o + hi - lo] = s_
                A3[kk, 1, o:o + hi - lo] = -2.0 * s_ * t
                A3[kk, 2, o:o + hi - lo] = s_ * t * t
            pair, sub = kk // 2, kk % 2
            o64 = order.astype(np.int64)
            iA = np.where(o64 < M // 2, o64, -1).astype(np.int16)
            iB = np.where(o64 >= M // 2, o64 - M // 2, -1).astype(np.int16)
            rows = slice(64 * sub, 64 * sub + 64)
            sidx[pair, 0, rows, :] = iA[None, :]
            sidx[pair, 1, rows, :] = iB[None, :]

        u = np.arange(-64, 64, dtype=np.float32)
        b3 = np.stack([u * u, u, np.ones(128, np.float32)])
        basis = np.zeros((99, 128), np.float32)
        for rg in range(4):
            basis[32 * rg:32 * rg + 3] = b3

        in_maps.append({
            "hTb": hTb, "Wv": Wv, "Wo": Wo,
            "A3": A3,
            "sidx": sidx, "basis": basis,
        })

    return in_maps, tuple(ranges)


def _build_kernel(ranges):
    import concourse.tile as tile
    from concourse import bacc, mybir
    from concourse.alu_op_type import AluOpType

    F32 = mybir.dt.float32
    BF16 = mybir.dt.bfloat16
    FP16 = mybir.dt.float16
    I16 = mybir.dt.int16

    nc = bacc.Bacc("TRN2", target_bir_lowering=False, debug=False, num_devices=NCORES)

    offs, cw = [], 0
    for lo, hi in ranges:
        offs.append(cw)
        cw += hi - lo
    hTb = nc.dram_tensor("hTb", [HS, M], BF16, kind="ExternalInput")
    Wv = nc.dram_tensor("Wv", [HS, CP], BF16, kind="ExternalInput")
    Wo = nc.dram_tensor("Wo", [CP, HS], BF16, kind="ExternalInput")
    A3 = nc.dram_tensor("A3", [HPC, 3, cw], F32, kind="ExternalInput")
    sidx = nc.dram_tensor("sidx", [2, 2, 128, M], I16, kind="ExternalInput")
    basis = nc.dram_tensor("basis", [99, 128], F32, kind="ExternalInput")
    out_part = nc.dram_tensor("out_part", [M, HS], BF16, kind="ExternalOutput")

    NC8 = HS // 128
    copy_ctr = [0]

    def copy_cast(out_ap, in_ap):
        if copy_ctr[0] % 2 == 0:
            nc.vector.tensor_copy(out_ap, in_ap)
        else:
            nc.scalar.copy(out_ap, in_ap)
        copy_ctr[0] += 1

    with tile.TileContext(nc) as tc:
        with (
            tc.tile_pool(name="persist", bufs=1) as persist,
            tc.tile_pool(name="vpool", bufs=1) as vpool,
            tc.tile_pool(name="attn_pool", bufs=14) as attn_pool,
            tc.tile_pool(name="a3_pool", bufs=2) as a3_pool,
            tc.tile_pool(name="norm_pool", bufs=4) as norm_pool,
            tc.tile_pool(name="out_pool", bufs=3) as out_pool,
            tc.tile_pool(name="ps", bufs=2, space="PSUM") as ps,
        ):
            basis_sb = persist.tile([99, 128], F32, name="basis")
            nc.sync.dma_start(basis_sb[:], basis[:])
            Wv_sb = []
            for c in range(NC8):
                t = persist.tile([128, CP], BF16, name=f"Wv{c}")
                nc.sync.dma_start(t[:], Wv[c * 128:(c + 1) * 128, :])
                Wv_sb.append(t)
            hT_sb = []
            for c in range(NC8):
                t = persist.tile([128, M], BF16, name=f"hT{c}")
                nc.sync.dma_start(t[:], hTb[c * 128:(c + 1) * 128, :])
                hT_sb.append(t)
            Wo_sb = []
            for p in range(2):
                t = persist.tile([128, HS], BF16, name=f"Wo{p}")
                nc.sync.dma_start(t[:], Wo[p * 128:(p + 1) * 128, :])
                Wo_sb.append(t)
            sidx_sb = []
            for p in range(2):
                row = []
                for hh in range(2):
                    t = persist.tile([128, M], I16, name=f"sidx{p}{hh}")
                    nc.sync.dma_start(t[:], sidx[p, hh])
                    row.append(t)
                sidx_sb.append(row)
            ones_sb = persist.tile([1, 64], FP16, name="ones64")
            nc.vector.memset(ones_sb[:], 1.0)

            pair_sb = [persist.tile([128, M], BF16, name=f"pair{p}") for p in range(2)]
            nat_sb = [persist.tile([128, M], BF16, name=f"nat{p}") for p in range(2)]

            # ---- value (per key block), with a ones column per head ----
            v_sb = []
            for jb in range(NJB):
                vt = vpool.tile([128, HPC * (D + 1)], BF16, name=f"v{jb}")
                pv = ps.tile([128, CP], F32, name="pv", tag="sc", bufs=3)
                for c in range(NC8):
                    for ch in range(2):
                        nc.tensor.matmul(
                            pv[64 * ch:64 * ch + 64, :],
                            hT_sb[c][:, jb * 128 + 64 * ch: jb * 128 + 64 * ch + 64],
                            Wv_sb[c][:],
                            start=(c == 0), stop=(c == NC8 - 1),
                            tile_position=(0, 64 * ch))
                for kk in range(HPC):
                    copy_cast(vt[:, kk * (D + 1):kk * (D + 1) + D],
                              pv[:, kk * D:(kk + 1) * D])
                    nc.vector.memset(vt[:, kk * (D + 1) + D:kk * (D + 1) + D + 1], 1.0)
                v_sb.append(vt)

            # ---- scores + attention per head ----
            rg_ctr = [0]
            HM = M // 2

            def emit_scores(kk, uh, A_t):
                h_lo, h_hi = uh * HM, (uh + 1) * HM
                chunks = []
                for jb in range(NJB):
                    lo, hi = ranges[jb]
                    ulo, uhi = max(lo, h_lo), min(hi, h_hi)
                    if uhi <= ulo:
                        continue
                    at = attn_pool.tile([128, 2 * SUB], BF16, name="at", tag="at")
                    segs = []
                    for q in range(2):
                        s0 = max(ulo, h_lo + q * SUB)
                        s1 = min(uhi, h_lo + (q + 1) * SUB)
                        if s1 <= s0:
                            continue
                        w = s1 - s0
                        rg = rg_ctr[0] % 4
                        rg_ctr[0] += 1
                        sc = ps.tile([128, SUB], F32, name="sc", tag="sc", bufs=3)
                        nc.tensor.matmul(
                            sc[:, :w], basis_sb[32 * rg:32 * rg + 3, :],
                            A_t[32 * rg:32 * rg + 3,
                                offs[jb] + s0 - lo: offs[jb] + s1 - lo],
                            start=True, stop=True, tile_position=(32 * rg, 0))
                        nc.scalar.activation(at[:, s0 - ulo:s1 - ulo], sc[:, :w],
                                             mybir.ActivationFunctionType.Exp,
                                             scale=-1.0)
                        segs.append((s0, s1, q))
                    chunks.append((at, jb, ulo, segs))
                return chunks

            def emit_attn(kk, uh, chunks, o_ps):
                h_lo = uh * HM
                bank_first = [True, True]
                for at, jb, ulo, segs in chunks:
                    vblk = v_sb[jb][:, kk * (D + 1):(kk + 1) * (D + 1)]
                    if all(not bank_first[q] for _, _, q in segs):
                        s0, s1 = segs[0][0], segs[-1][1]
                        nc.tensor.matmul(
                            o_ps[:, s0 - h_lo:s1 - h_lo], vblk,
                            at[:, s0 - ulo:s1 - ulo], start=False, stop=False)
                    else:
                        for s0, s1, q in segs:
                            nc.tensor.matmul(
                                o_ps[:, s0 - h_lo:s1 - h_lo], vblk,
                                at[:, s0 - ulo:s1 - ulo],
                                start=bank_first[q], stop=False)
                            bank_first[q] = False

            def emit_norm_act(prev):
                kk, uh, o_ps = prev
                rcrs = []
                for q in range(2):
                    rcr = norm_pool.tile([1, SUB], FP16, name="rcr", tag="rcr")
                    nc.scalar.copy(rcr[:], o_ps[64:65, q * SUB:(q + 1) * SUB])
                    rcrs.append(rcr)
                return rcrs

            def emit_norm_rest(prev, rcrs):
                kk, uh, o_ps = prev
                pair, sub = kk // 2, kk % 2
                h_lo = uh * HM
                for q in range(2):
                    qs = slice(q * SUB, (q + 1) * SUB)
                    gqs = slice(h_lo + q * SUB, h_lo + (q + 1) * SUB)
                    bc = ps.tile([64, SUB], F32, name="bc", tag="bc", bufs=1)
                    nc.tensor.matmul(bc[:], ones_sb[:], rcrs[q][:], start=True, stop=True)
                    rcs = norm_pool.tile([64, SUB], F32, name="rcs", tag="rcs")
                    nc.vector.reciprocal_approx_fast(rcs[:], bc[:])
                    nc.vector.tensor_tensor(
                        pair_sb[pair][64 * sub:64 * sub + 64, gqs],
                        o_ps[0:64, qs], rcs[:], AluOpType.mult)

            prev = None
            prev_rcrs = None
            A_tiles = {}
            for kk in range(HPC):
                A_t = a3_pool.tile([99, cw], F32, name="At", tag="At")
                for rg in range(4):
                    nc.sync.dma_start(A_t[32 * rg:32 * rg + 3, :], A3[kk])
                for uh in range(2):
                    if prev is not None:
                        prev_rcrs = emit_norm_act(prev)
                    chunks = emit_scores(kk, uh, A_t)
                    if prev is not None:
                        emit_norm_rest(prev, prev_rcrs)
                    o_ps = ps.tile([65, HM], F32, name="oT", tag="oT", bufs=2)
                    emit_attn(kk, uh, chunks, o_ps)
                    prev = (kk, uh, o_ps)
            prev_rcrs = emit_norm_act(prev)
            emit_norm_rest(prev, prev_rcrs)

            # ---- un-permute sorted -> natural (gpsimd) ----
            for p in range(2):
                for hh in range(2):
                    nc.gpsimd.local_scatter(
                        nat_sb[p][:, hh * (M // 2):(hh + 1) * (M // 2)],
                        pair_sb[p][:], sidx_sb[p][hh][:],
                        channels=128, num_elems=M // 2, num_idxs=M)

            # ---- output projection ----
            for ic in range(M // 128):
                ics = slice(ic * 128, (ic + 1) * 128)
                ot = out_pool.tile([128, HS], BF16, name="ot", tag="ot")
                for jh in range(2):
                    jhs = slice(jh * 512, (jh + 1) * 512)
                    pp = ps.tile([128, 512], F32, name="pp", tag="sc", bufs=3)
                    nc.tensor.matmul(pp[:], nat_sb[0][:, ics], Wo_sb[0][:, jhs],
                                     start=True, stop=False)
                    nc.tensor.matmul(pp[:], nat_sb[1][:, ics], Wo_sb[1][:, jhs],
                                     start=False, stop=True)
                    copy_cast(ot[:, jhs], pp[:])
                nc.sync.dma_start(out_part[ics, :], ot[:])

    nc.compile()
    return nc


def kernel(h, W_span, W_val, W_out):
    _ensure_ntff_hook()
    from concourse.bass_utils import run_bass_kernel_spmd

    in_maps, ranges = _build_host_data(h, W_span, W_val, W_out)
    nc = _CACHE.get(ranges)
    if nc is None:
        nc = _build_kernel(ranges)
        _CACHE[ranges] = nc

    res = run_bass_kernel_spmd(nc, in_maps, list(range(NCORES)), trace=False)

    out = np.zeros((B, M, HS), np.float32)
    for core in range(NCORES):
        out[core // HPC] += res.results[core]["out_part"].astype(np.float32)
    return out



# revision 3
# speedup vs baseline: 1.3195x; 1.3195x over previous
"""Trainium2 Bass kernel for the Gaussian-span multi-head self-attention module.

  span  = head_reshape(h @ W_span.T, 2)          (B*K, M, 2)
  value = head_reshape(h @ W_val.T, D)           (B*K, M, D)
  mean  = sigmoid(span0) * M ; soft = softplus(span1)
  attn  = softmax(-soft * (pos - mean)^2)        (B*K, M, M)
  out   = (attn @ value)  -> concat heads -> @ W_out.T

Shapes are hardcoded: B=2, M=2048, HS=1024, K=16 heads, D=64.

Strategy (8 NeuronCores, SPMD - one program, per-core data):
  * batch*head sharding: each core handles one batch and 4 head instances,
    assigned by a host-side greedy clustering that minimizes the shared
    windowed-schedule width per program slot.
  * Host computes the tiny span projection, sorts each head's query rows by
    their Gaussian mean, and builds a per-slot windowed schedule: for each
    128-wide key block only the contiguous range of sorted rows whose
    Gaussian window (tail exp(-9)) touches it is processed.
  * Scores are two concurrent rank-6 fp16 matmuls per 128-key block (the two
    64-wide halves packed into PE row/col groups (0,0) and (32,64)); the
    quadratic -s(u-t)^2 is expanded against a per-64-block-centered basis
    [u^2,u,1] with hi/lo-split fp16 coefficients, so scores are exact to
    ~3e-3 in f32 PSUM.  One ScalarE exp(-x) per 512-wide PSUM chunk.
  * attn @ value accumulates out^T (65 x 1024) in PSUM per row-half with the
    fp16 value tile stationary; a ones-column yields the softmax denominator.
  * Value is computed per key block (stationary hT block, moving fp16 W_val
    slice) and pipelined against the hT DMA, which streams in 256-column
    chunks; the attention strips interleave with the value matmuls.
  * The sorted->natural un-permutation runs on GPSIMD local_scatter per
    head-pair/dest-half, overlapped with later strips and the output
    projection, which is interleaved per natural half.
"""

import sys
import types

import numpy as np
import ml_dtypes

B, M, HS, NH, D = 2, 2048, 1024, 16, 64
NCORES = 8
HPC = 4            # head slots per core
CP = HPC * D       # 256 channels per core
NJB = M // 128     # 128-wide key blocks
HM = M // 2        # row half
TAIL_T = 9.0       # window cut: dropped tail mass ~ exp(-9) ~ 1.2e-4
ALIGN = 4

_CACHE = {}


def _ensure_ntff_hook():
    """Install the antenv.axon_hooks shim if the image lacks it (profiling only)."""
    try:
        import antenv.axon_hooks  # noqa: F401
        return
    except ImportError:
        pass
    try:
        import antenv
        from trn_agent_boot.trn_boot import _ntff_profile_via_ctypes
    except ImportError:
        return
    mod = types.ModuleType("antenv.axon_hooks")
    _h = [None]
    mod.set_axon_ntff_profile_hook = lambda hk: _h.__setitem__(0, hk)
    mod.get_axon_ntff_profile_hook = lambda: _h[0]
    sys.modules["antenv.axon_hooks"] = mod
    antenv.axon_hooks = mod
    try:
        mod.set_axon_ntff_profile_hook(
            _ntff_profile_via_ctypes("/opt/axon/libaxon_pjrt.so"))
    except Exception:
        pass


def _sigmoid64(x):
    return 1.0 / (1.0 + np.exp(-x.astype(np.float64)))


def _softplus64(x):
    return np.logaddexp(0.0, x.astype(np.float64))


def _band(ms, ws):
    """Per 128-block [ilo, ihi) over sorted rows whose window touches it."""
    lo, hi = ms - ws, ms + ws
    ilos = np.full(NJB, M, np.int64)
    ihis = np.zeros(NJB, np.int64)
    for jb in range(NJB):
        mask = (hi >= jb * 128) & (lo <= jb * 128 + 128)
        idx = np.flatnonzero(mask)
        if idx.size:
            ilos[jb] = idx[0]
            ihis[jb] = idx[-1] + 1
    return ilos, ihis


def _assign_slots(bands):
    """Greedy: assign instances (b,k) to 4 slots (4 per batch each),
    minimizing the summed envelope width."""
    insts = [(b, k) for b in range(B) for k in range(NH)]
    width = {bk: int((bands[bk][1] - np.minimum(bands[bk][0], bands[bk][1])).sum())
             for bk in insts}
    insts.sort(key=lambda bk: -width[bk])
    slot_lo = [np.full(NJB, M, np.int64) for _ in range(HPC)]
    slot_hi = [np.zeros(NJB, np.int64) for _ in range(HPC)]
    slot_cnt = [[0, 0] for _ in range(HPC)]
    assign = {}

    def cost(lo, hi):
        return int(np.maximum(hi - lo, 0).sum())

    for bk in insts:
        ilo, ihi = bands[bk]
        best, bestd = None, None
        for s in range(HPC):
            if slot_cnt[s][bk[0]] >= B * 2:
                continue
            nlo = np.minimum(slot_lo[s], ilo)
            nhi = np.maximum(slot_hi[s], ihi)
            d = cost(nlo, nhi) - cost(slot_lo[s], slot_hi[s])
            if bestd is None or d < bestd:
                best, bestd = s, d
        s = best
        slot_lo[s] = np.minimum(slot_lo[s], ilo)
        slot_hi[s] = np.maximum(slot_hi[s], ihi)
        slot_cnt[s][bk[0]] += 1
        assign[bk] = s
    return assign, slot_lo, slot_hi


def _build_sched(slot_lo, slot_hi):
    """Per-slot, per-half segment/chunk schedule."""
    sched = []
    for s in range(HPC):
        ranges = []
        for jb in range(NJB):
            lo, hi = int(slot_lo[s][jb]), int(slot_hi[s][jb])
            if hi <= lo:
                ranges.append((0, 0))
            else:
                ranges.append((lo & ~(ALIGN - 1),
                               min(M, (hi + ALIGN - 1) & ~(ALIGN - 1))))
        halves = []
        for uh in range(2):
            h_lo = uh * HM
            segs = []
            off = 0
            for jb in range(NJB):
                lo, hi = ranges[jb]
                s0, s1 = max(lo, h_lo), min(hi, h_lo + HM)
                if s1 <= s0:
                    continue
                segs.append((jb, s0, s1, off))
                off += s1 - s0
            cw = off
            # split segs at packed-512 and (s-h_lo)%512 boundaries
            pieces = []
            for jb, s0, s1, o0 in segs:
                cur = s0
                while cur < s1:
                    o = o0 + (cur - s0)
                    nxt = min(s1,
                              cur + (512 - (o % 512)),
                              h_lo + ((cur - h_lo) // 512 + 1) * 512)
                    pieces.append((jb, cur, nxt, o))
                    cur = nxt
            nchunks = (cw + 511) // 512
            chunks = []
            for ci in range(nchunks):
                c0, c1 = ci * 512, min(cw, (ci + 1) * 512)
                ps = [p for p in pieces if c0 <= p[3] < c1]
                need_jb = max(p[0] for p in ps)
                chunks.append({"c0": c0, "c1": c1, "pieces": ps,
                               "need_jb": need_jb})
            # last piece per o_ps bank (for stop flag)
            lastp = {}
            for i, p in enumerate(pieces):
                lastp[(p[1] - h_lo) // 512] = i
            halves.append({"cw": cw, "segs": segs, "pieces": pieces,
                           "chunks": chunks,
                           "last_by_bank": set(lastp.values())})
        sched.append({"ranges": tuple(ranges), "halves": halves})
    return sched


def _f16_split(x):
    hi = x.astype(np.float16)
    lo = (x - hi.astype(np.float64)).astype(np.float16)
    return hi, lo


def _build_host_data(h, W_span, W_val, W_out):
    h = np.asarray(h, np.float32)
    W_span = np.asarray(W_span, np.float32)
    W_val = np.asarray(W_val, np.float32)
    W_out = np.asarray(W_out, np.float32)

    span = (h.reshape(B * M, HS) @ W_span.T).reshape(B, M, 2 * NH)

    m_all = np.zeros((B, NH, M), np.float64)
    s_all = np.zeros((B, NH, M), np.float64)
    for b in range(B):
        for k in range(NH):
            m_all[b, k] = _sigmoid64(span[b, :, 2 * k]) * M
            s_all[b, k] = _softplus64(span[b, :, 2 * k + 1])
    order_all = np.argsort(m_all, axis=-1, kind="stable")
    W_all = np.sqrt(TAIL_T / np.maximum(s_all, 1e-12))

    bands = {}
    for b in range(B):
        for k in range(NH):
            o = order_all[b, k]
            bands[(b, k)] = _band(m_all[b, k][o], W_all[b, k][o])
    assign, slot_lo, slot_hi = _assign_slots(bands)
    sched = _build_sched(slot_lo, slot_hi)

    # coverage: every sorted row must fall in the range of its own mean block
    for (b, k), s in assign.items():
        ranges = sched[s]["ranges"]
        ms = m_all[b, k][order_all[b, k]]
        own = np.clip((ms // 128).astype(np.int64), 0, NJB - 1)
        pos = np.arange(M)
        lows = np.array([ranges[j][0] for j in own])
        highs = np.array([ranges[j][1] for j in own])
        if not ((lows <= pos) & (pos < highs)).all():
            raise AssertionError("window schedule does not cover all rows")

    # core (b, g) takes 4 instances of batch b, one per slot
    per_slot_heads = [[[], []] for _ in range(HPC)]
    for (b, k), s in assign.items():
        per_slot_heads[s][b].append(k)

    cwmax = max(sched[s]["halves"][uh]["cw"] for s in range(HPC)
                for uh in range(2))
    cwmax = (cwmax + 7) & ~7

    u = np.arange(-32, 32, dtype=np.float64)
    u2 = (u * u).astype(np.float16).astype(np.float64)
    basis = np.zeros((38, 64), np.float16)
    for base in (0, 32):
        basis[base + 0] = u2
        basis[base + 1] = u
        basis[base + 2] = 1.0
        basis[base + 3] = u2
        basis[base + 4] = u
        basis[base + 5] = 1.0

    in_maps = []
    for core in range(NCORES):
        b, g = core // HPC, core % HPC
        heads = [per_slot_heads[s][b][g] for s in range(HPC)]

        hT = np.ascontiguousarray(
            h[b].T.reshape(8, 128, M).transpose(1, 0, 2)).astype(np.float16)
        chans = np.concatenate([np.arange(k * D, (k + 1) * D) for k in heads])
        Wv = np.ascontiguousarray(
            W_val[chans, :].T.reshape(8, 128, CP).transpose(1, 0, 2)
        ).astype(np.float16)
        Wo = np.ascontiguousarray(
            W_out[:, chans].T.reshape(2, 128, HS).transpose(1, 0, 2)
        ).astype(np.float16)

        A6 = np.zeros((HPC, 2, 12, cwmax), np.float16)
        sidx = np.zeros((128, 4, M), np.int16)
        for kk, k in enumerate(heads):
            o = order_all[b, k]
            ms = m_all[b, k][o]
            ss = s_all[b, k][o]
            for uh in range(2):
                for jb, s0, s1, off in sched[kk]["halves"][uh]["segs"]:
                    mseg, sseg = ms[s0:s1], ss[s0:s1]
                    n = s1 - s0
                    for par, center in ((0, 128 * jb + 32), (1, 128 * jb + 96)):
                        t = mseg - center
                        s_ = sseg.copy()
                        c1 = -2.0 * sseg * t
                        c0 = sseg * t * t
                        # rows far outside this 64-block: flat huge score
                        # (weight exp(-x) == 0 either way; avoids fp16 overflow)
                        far = c0 > 50000.0
                        s_[far] = 0.0
                        c1[far] = 0.0
                        c0[far] = 50000.0
                        sh, sl = _f16_split(s_)
                        c1h, c1l = _f16_split(c1)
                        c0h, c0l = _f16_split(c0)
                        rows = A6[kk, uh, 6 * par:6 * par + 6, off:off + n]
                        rows[0], rows[1], rows[2] = sh, c1h, c0h
                        rows[3], rows[4], rows[5] = sl, c1l, c0l
            p, sub = kk // 2, kk % 2
            o64 = o.astype(np.int64)
            for hh in range(2):
                arr = np.where((o64 >= hh * HM) & (o64 < (hh + 1) * HM),
                               o64 - hh * HM, -1).astype(np.int16)
                sidx[64 * sub:64 * sub + 64, 2 * p + hh, :] = arr[None, :]

        in_maps.append({
            "hT": hT, "Wv": Wv, "Wo": Wo, "A6": A6,
            "sidx": sidx, "basis": basis,
        })

    key = tuple(sched[s]["ranges"] for s in range(HPC)) + (cwmax,)
    return in_maps, key, sched, cwmax


def _build_kernel(sched, cwmax):
    import concourse.tile as tile
    from concourse import bacc, mybir
    from concourse.alu_op_type import AluOpType

    F32 = mybir.dt.float32
    F16 = mybir.dt.float16
    I16 = mybir.dt.int16

    nc = bacc.Bacc("TRN2", target_bir_lowering=False, debug=False,
                   num_devices=NCORES)

    hT = nc.dram_tensor("hT", [128, 8, M], F16, kind="ExternalInput")
    Wv = nc.dram_tensor("Wv", [128, 8, CP], F16, kind="ExternalInput")
    Wo = nc.dram_tensor("Wo", [128, 2, HS], F16, kind="ExternalInput")
    A6 = nc.dram_tensor("A6", [HPC, 2, 12, cwmax], F16, kind="ExternalInput")
    sidx = nc.dram_tensor("sidx", [128, 4, M], I16, kind="ExternalInput")
    basis = nc.dram_tensor("basis", [38, 64], F16, kind="ExternalInput")
    out_part = nc.dram_tensor("out_part", [M, HS], F16, kind="ExternalOutput")

    with tile.TileContext(nc) as tc:
        with (
            tc.tile_pool(name="persist", bufs=1) as persist,
            tc.tile_pool(name="at_pool", bufs=6) as at_pool,
            tc.tile_pool(name="norm_pool", bufs=4) as norm_pool,
            tc.tile_pool(name="out_pool", bufs=3) as out_pool,
            tc.tile_pool(name="ps", bufs=2, space="PSUM") as ps,
        ):
            # ---- persistent tiles ----
            basis_sb = persist.tile([38, 64], F16, name="basis")
            hT_sb = persist.tile([128, 8, M], F16, name="hT")
            Wv_sb = persist.tile([128, 8, CP], F16, name="Wv")
            Wo_sb = persist.tile([128, 2, HS], F16, name="Wo")
            sidx_sb = persist.tile([128, 4, M], I16, name="sidx")
            A6_sb = [[persist.tile([38, max(sched[kk]["halves"][uh]["cw"], 8)],
                                   F16, name=f"A6_{kk}_{uh}")
                      for uh in range(2)] for kk in range(HPC)]
            v_sb = [persist.tile([128, HPC, D + 1], F16, name=f"v{jb}")
                    for jb in range(NJB)]
            pair_sb = [persist.tile([128, M], F16, name=f"pair{p}")
                       for p in range(2)]
            nat_sb = [persist.tile([128, M], F16, name=f"nat{p}")
                      for p in range(2)]
            ones_sb = persist.tile([1, 64], F16, name="ones64")
            actw_sb = persist.tile([1, 16], F32, name="actw")
            actw_o = persist.tile([1, 16], F16, name="actwo")

            # ---- activation table preload (scalar queue head) ----
            nc.vector.memset(actw_sb[:], 1.0)
            nc.scalar.activation(actw_o[:], actw_sb[:],
                                 mybir.ActivationFunctionType.Exp, scale=-1.0)

            # ---- input DMA (sync queue); A6 strips on scalar queue ----
            nc.sync.dma_start(basis_sb[:], basis[:])
            nc.sync.dma_start(Wv_sb[:], Wv[:])
            for kk in range(HPC):
                for uh in range(2):
                    cw = sched[kk]["halves"][uh]["cw"]
                    if cw == 0:
                        continue
                    eng = nc.sync if (kk, uh) == (0, 0) else nc.scalar
                    eng.dma_start(A6_sb[kk][uh][0:6, :cw], A6[kk, uh, 0:6, :cw])
                    eng.dma_start(A6_sb[kk][uh][32:38, :cw], A6[kk, uh, 6:12, :cw])
            for jc in range(8):
                cs = slice(jc * 256, (jc + 1) * 256)
                nc.sync.dma_start(hT_sb[:, :, cs], hT[:, :, cs])
            nc.sync.dma_start(sidx_sb[:], sidx[:])
            nc.sync.dma_start(Wo_sb[:], Wo[:])
            nc.vector.memset(ones_sb[:], 1.0)

            # ---- PE warmup: release the HAM throttle during initial DMA ----
            warm = ps.tile([64, 64], F32, name="warm", tag="pv", bufs=1)
            for _ in range(48):
                nc.tensor.matmul(warm[:], basis_sb[0:6, :], basis_sb[0:6, :],
                                 start=True, stop=True, tile_position=(0, 0))

            # ---- value per key block ----
            def emit_value(jb):
                pv = ps.tile([128, HPC, D], F32, name="pv", tag="pv", bufs=1)
                for c in range(8):
                    nc.tensor.matmul(
                        pv[:], hT_sb[:, c, jb * 128:(jb + 1) * 128],
                        Wv_sb[:, c, :],
                        start=(c == 0), stop=(c == 7))
                nc.vector.tensor_copy(v_sb[jb][:, :, 0:D], pv[:])
                nc.vector.memset(v_sb[jb][:, :, D:D + 1], 1.0)

            # ---- one attention strip (kk, uh) ----
            def emit_strip(kk, uh, jb_limit=None, after2=None):
                """Emit scores/exp/attn for strip; returns o_ps or None.
                jb_limit: generator yielding chunks gated by value progress."""
                H = sched[kk]["halves"][uh]
                cw = H["cw"]
                h_lo = uh * HM
                if cw == 0:
                    return None
                A6t = A6_sb[kk][uh]
                o_ps = ps.tile([65, HM], F32, name="oT", tag="oT", bufs=2)
                bank_first = [True, True]
                pend = []          # (at_tile, chunk) awaiting attn
                done2 = [False]

                def flush_one():
                    at_t, ch = pend.pop(0)
                    for idx, (jb, s0, s1, off) in enumerate(ch["pieces"]):
                        pi = H["pieces"].index((jb, s0, s1, off))
                        q = (s0 - h_lo) // 512
                        nc.tensor.matmul(
                            o_ps[:, s0 - h_lo:s1 - h_lo],
                            v_sb[jb][:, kk, :],
                            at_t[:, off - ch["c0"]:off - ch["c0"] + (s1 - s0)],
                            start=bank_first[q],
                            stop=(pi in H["last_by_bank"]))
                        bank_first[q] = False

                for ci, ch in enumerate(H["chunks"]):
                    w = ch["c1"] - ch["c0"]
                    sc = ps.tile([128, 512], F32, name="sc", tag="sc", bufs=3)
                    for jb, s0, s1, off in ch["pieces"]:
                        r0 = off - ch["c0"]
                        n = s1 - s0
                        nc.tensor.matmul(
                            sc[0:64, r0:r0 + n], basis_sb[0:6, :],
                            A6t[0:6, off:off + n],
                            start=True, stop=True, tile_position=(0, 0))
                        nc.tensor.matmul(
                            sc[64:128, r0:r0 + n], basis_sb[32:38, :],
                            A6t[32:38, off:off + n],
                            start=True, stop=True, tile_position=(32, 64))
                    at_t = at_pool.tile([128, 512], F16, name="at", tag="at")
                    nc.scalar.activation(at_t[:, :w], sc[:, :w],
                                         mybir.ActivationFunctionType.Exp,
                                         scale=-1.0)
                    pend.append((at_t, ch))
                    if len(pend) > 2:
                        flush_one()
                    if ci == 1 and after2 is not None:
                        after2()
                        done2[0] = True
                while pend:
                    flush_one()
                if not done2[0] and after2 is not None:
                    after2()
                return o_ps

            # ---- normalization of a finished strip ----
            def emit_norm(kk, uh, o_ps):
                if o_ps is None:
                    return
                p, sub = kk // 2, kk % 2
                h_lo = uh * HM
                for q in range(2):
                    qs = slice(q * 512, (q + 1) * 512)
                    rcr = norm_pool.tile([1, 512], F16, name="rcr", tag="rcr")
                    nc.vector.tensor_copy(rcr[:], o_ps[64:65, qs])
                    bc = ps.tile([64, 512], F32, name="bc", tag="sc", bufs=3)
                    nc.tensor.matmul(bc[:], ones_sb[:], rcr[:],
                                     start=True, stop=True)
                    rcs = norm_pool.tile([64, 512], F32, name="rcs", tag="rcs")
                    nc.vector.reciprocal_approx_fast(rcs[:], bc[:])
                    nc.vector.tensor_tensor(
                        pair_sb[p][64 * sub:64 * sub + 64,
                                   h_lo + q * 512:h_lo + (q + 1) * 512],
                        o_ps[0:64, qs], rcs[:], AluOpType.mult)

            # ---- phase A: value interleaved with strip (0,0) ----
            H00 = sched[0]["halves"][0]
            chunks00 = H00["chunks"]
            state = {"o_ps": None, "next_chunk": 0}

            o_ps00 = ps.tile([65, HM], F32, name="oT", tag="oT", bufs=2)
            bank_first00 = [True, True]
            pend00 = []

            def flush00():
                at_t, ch = pend00.pop(0)
                for jb, s0, s1, off in ch["pieces"]:
                    pi = H00["pieces"].index((jb, s0, s1, off))
                    q = s0 // 512
                    nc.tensor.matmul(
                        o_ps00[:, s0:s1], v_sb[jb][:, 0, :],
                        at_t[:, off - ch["c0"]:off - ch["c0"] + (s1 - s0)],
                        start=bank_first00[q],
                        stop=(pi in H00["last_by_bank"]))
                    bank_first00[q] = False

            def emit_chunk00(ch):
                w = ch["c1"] - ch["c0"]
                sc = ps.tile([128, 512], F32, name="sc", tag="sc", bufs=3)
                for jb, s0, s1, off in ch["pieces"]:
                    r0 = off - ch["c0"]
                    n = s1 - s0
                    nc.tensor.matmul(sc[0:64, r0:r0 + n], basis_sb[0:6, :],
                                     A6_sb[0][0][0:6, off:off + n],
                                     start=True, stop=True, tile_position=(0, 0))
                    nc.tensor.matmul(sc[64:128, r0:r0 + n], basis_sb[32:38, :],
                                     A6_sb[0][0][32:38, off:off + n],
                                     start=True, stop=True,
                                     tile_position=(32, 64))
                at_t = at_pool.tile([128, 512], F16, name="at", tag="at")
                nc.scalar.activation(at_t[:, :w], sc[:, :w],
                                     mybir.ActivationFunctionType.Exp,
                                     scale=-1.0)
                pend00.append((at_t, ch))
                if len(pend00) > 2:
                    flush00()

            for jc in range(8):
                emit_value(2 * jc)
                emit_value(2 * jc + 1)
                while (state["next_chunk"] < len(chunks00) and
                       chunks00[state["next_chunk"]]["need_jb"] <= 2 * jc + 1):
                    emit_chunk00(chunks00[state["next_chunk"]])
                    state["next_chunk"] += 1
            while state["next_chunk"] < len(chunks00):
                emit_chunk00(chunks00[state["next_chunk"]])
                state["next_chunk"] += 1
            while pend00:
                flush00()

            # ---- phase B: remaining strips, norms, scatters, out-proj ----
            strip_list = [(0, 1), (1, 0), (1, 1), (2, 0), (2, 1),
                          (3, 0), (3, 1)]
            prev = (0, 0, o_ps00)

            def scatter_pair(p):
                for hh in range(2):
                    nc.gpsimd.local_scatter(
                        nat_sb[p][:, hh * HM:(hh + 1) * HM],
                        pair_sb[p][:], sidx_sb[:, 2 * p + hh, :],
                        channels=128, num_elems=HM, num_idxs=M)

            for (kk, uh) in strip_list:
                pk, pu, po = prev
                o_ps = emit_strip(kk, uh,
                                  after2=(lambda pk=pk, pu=pu, po=po:
                                          emit_norm(pk, pu, po)))
                if (pk, pu) == (1, 1):
                    scatter_pair(0)
                prev = (kk, uh, o_ps)
            emit_norm(*prev)
            scatter_pair(1)

            # ---- output projection, interleaved per natural half ----
            for hh in range(2):
                for ic in range(hh * 8, hh * 8 + 8):
                    ics = slice(ic * 128, (ic + 1) * 128)
                    ot = out_pool.tile([128, HS], F16, name="ot", tag="ot")
                    for jh in range(2):
                        jhs = slice(jh * 512, (jh + 1) * 512)
                        pp = ps.tile([128, 512], F32, name="pp", tag="sc",
                                     bufs=3)
                        nc.tensor.matmul(pp[:], nat_sb[0][:, ics],
                                         Wo_sb[:, 0, jhs],
                                         start=True, stop=False)
                        nc.tensor.matmul(pp[:], nat_sb[1][:, ics],
                                         Wo_sb[:, 1, jhs],
                                         start=False, stop=True)
                        nc.vector.tensor_copy(ot[:, jhs], pp[:])
                    nc.scalar.dma_start(out_part[ics, :], ot[:])

    nc.compile()
    return nc


def kernel(h, W_span, W_val, W_out):
    _ensure_ntff_hook()
    from concourse.bass_utils import run_bass_kernel_spmd

    in_maps, key, sched, cwmax = _build_host_data(h, W_span, W_val, W_out)
    nc = _CACHE.get(key)
    if nc is None:
        nc = _build_kernel(sched, cwmax)
        _CACHE[key] = nc

    res = run_bass_kernel_spmd(nc, in_maps, list(range(NCORES)), trace=False)

    out = np.zeros((B, M, HS), np.float32)
    for core in range(NCORES):
        out[core // HPC] += res.results[core]["out_part"].astype(np.float32)
    return out
